# revision 43
# baseline (speedup 1.0000x reference)
"""Trainium2 Bass kernel for CompositionalTwoArmedAgent (DND-LSTM A2C step).

Strategy (8 NeuronCores, SPMD, ZERO collectives):
  - DND vals table sharded COLUMN-wise: core k owns h-dims [128k, 128k+128)
    end-to-end -- its m_t slice, its 640 W_h2h rows, its gates, and its
    h_t / c_t slices.  No cross-core dependency, so no AllReduce: on this
    tunneled runtime the collective stack costs ~100us (entry barrier +
    33us mesh AR for 25KB), dwarfing the ~40us of real work.
  - Cosine sims are host-folded (like the baseline's key-norm fold): the
    device gets max-subtracted dots, does exp -> rowsum -> full-sum (two
    tiny PE matmuls; no gpsimd) -> fp8 DoubleRow weights.  dots arrive in
    two halves so exp/cast pipeline with the DMA.
  - vals stream in fp8e4 DoubleRow pairs (2 row-chunks of 128/partition):
    391 matmuls of FD=128, DMA-bound at ~350GB/s for 12.8MB/core.  The
    sync HWDGE queue carries ONLY the vals blocks so the stream starts at
    the queue-open tick; blocks taper at the end to cut the PE tail-lag.
  - LSTM preact slice = W_h2h[rows] @ h accumulated over 8 h-chunks; the
    x_t @ W_i2h part folds into the bias on the host.  The whole preact/
    gate chain is pushed late in the Tile schedule (tile_wait_until) so it
    gap-fills the PE instead of blocking the DR stream behind the slower
    wht DMA.
  - A2C: each core emits q_k = W_ih[:, cols_k] @ h_t[cols_k] (all GEMM
    flops stay on device); the host sums the 8 partials, applies
    relu/actor/critic/softmax/sampling -- same kind of post-processing the
    row-sharded baseline already did.
"""

import ml_dtypes
import numpy as np

import concourse.bacc as bacc
import concourse.bass as bass
import concourse.mybir as mybir
import concourse.tile as tile
from concourse.bass_utils import run_bass_kernel_spmd

N_CORES = 8
D, RD, H, IN_DIM = 100000, 10, 1024, 14
CH = 782               # 128-row chunks over D (padded to 100096)
NG = 391               # DoubleRow chunk pairs
NGA = 196              # pairs covered by the first dots half
PAD_D = CH * 128
# vals DMA blocks all ride the sync HWDGE queue (the two HWDGE queues share
# one SDMA engine pool, so splitting them halves per-queue rate).  DMA rate
# is per-partition-line-size bound: 64-pair blocks = 16KB lines sustain
# ~430GB/s while small blocks collapse to 80-250GB/s, so blocks are uniform
# 64 with one small remainder at the end (short wire + short PE lag).
NB0 = 32               # pairs fused into the blk0 transfer with the weights
E8OFF = 784            # e8t bytes (782) padded to 16B alignment in blk0
BLOCKS = [32, 32, 64, 64, 64, 32, 32, 32, 7]   # remaining pair blocks
assert NB0 + sum(BLOCKS) == NG
F32 = mybir.dt.float32
F8 = mybir.dt.float8e4
BF16 = mybir.dt.bfloat16
F16 = mybir.dt.float16

# jax.random.gumbel(jax.random.key(1), (2,), float32) — fixed constants of the
# reference's categorical sample (verified against jax.random.categorical).
GUMBEL = np.array([0.5325072, -0.01641824], np.float32)

_CACHE = {}


def _input_specs():
    return [
        # [exp(dots-max) fp8 tight (782B, padded to 784) | first NB0 pairs of
        # vals] fused into one big-line transfer so the weights ride at full
        # DMA rate instead of 25GB/s small-line rate.
        ("blk0", [128, E8OFF + NB0 * 256], F8),
        ("vals_s", [128, sum(BLOCKS) * 256], F8),  # remaining vals pairs
        ("wht", [128, 8 * 640], F16),         # W_h2h rows for this col slice
        ("h8", [128, 8], F16),                # full h, chunked
        ("b5t", [128, 5], F32),               # b_i2h + b_h2h + W_i2h @ x slice
        ("c2t", [128, 1], F32),               # c slice
        ("wihs", [128, 1024], BF16),          # W_ih[:, cols_k].T
    ]


def _build():
    nc = bacc.Bacc("TRN2", target_bir_lowering=False, debug=False,
                   num_devices=1)
    d = {name: nc.dram_tensor(name, shp, dt, kind="ExternalInput")
         for name, shp, dt in _input_specs()}
    # [h_t | c_t | q partial as [128, 8]] -- one DMA, 40B lines
    out_hc = nc.dram_tensor("out_hc", [128, 10], F32, kind="ExternalOutput")

    AF = mybir.ActivationFunctionType
    OP = mybir.AluOpType
    PM = mybir.MatmulPerfMode

    with tile.TileContext(nc) as tc:
        with (
            tc.tile_pool(name="const", bufs=1) as cp,
            tc.tile_pool(name="vals", bufs=3) as vp,
            tc.tile_pool(name="ps", bufs=1, space="PSUM") as pp,
        ):
            # ---- persistent loads, split by queue for priority ----------
            blk0_sb = cp.tile([128, E8OFF + NB0 * 256], F8)
            wht_sb = cp.tile([128, 8, 640], F16)
            h8_sb = cp.tile([128, 8], F16)
            b5t_sb = cp.tile([128, 5], F32)
            c2t_sb = cp.tile([128, 1], F32)
            wihs_sb = cp.tile([128, 1024], BF16)
            # fused weights+first-vals transfer heads the sync HWDGE queue
            nc.sync.dma_start(blk0_sb[:], d["blk0"][:])
            # preact weights on the otherwise-empty scalar HWDGE queue so
            # they land ~12us and the early preact slot can never stall
            nc.scalar.dma_start(
                wht_sb[:], d["wht"][:].rearrange("p (c m) -> p c m", m=640))
            nc.scalar.dma_start(h8_sb[:], d["h8"][:])
            # tail-only tensors on the gpsimd SWDGE queue (idle engine)
            nc.gpsimd.dma_start(b5t_sb[:], d["b5t"][:])
            nc.gpsimd.dma_start(c2t_sb[:], d["c2t"][:])
            nc.gpsimd.dma_start(wihs_sb[:], d["wihs"][:])

            one16 = cp.tile([1, 1], F16)
            nc.vector.memset(one16[:], 1.0)
            ones1 = cp.tile([1, 1], F32)
            nc.vector.memset(ones1[:], 1.0)
            ones_col = cp.tile([128, 1], F32)
            nc.vector.memset(ones_col[:], 1.0)
            ones_row = cp.tile([1, 128], F32)
            nc.vector.memset(ones_row[:], 1.0)

            # ---- softmax numerator weights: scatter into the DoubleRow
            # 16B-strided stationary layout (halves, so DR g=0 starts early)
            e8t_v = blk0_sb[:, 0:CH]
            e8 = cp.tile([128, 2, 16 * NG], F8)
            e8v = e8[:].rearrange("p j (g s) -> p j g s", s=16)
            erv = e8t_v.rearrange("p (g j one) -> p j g one", j=2, one=1)
            nc.vector.tensor_copy(e8v[:, :, 0:NGA, 0:1], erv[:, :, 0:NGA, :])
            nc.vector.tensor_copy(e8v[:, :, NGA:NG, 0:1], erv[:, :, NGA:NG, :])
            rowsum = cp.tile([128, 1], F32)
            nc.vector.tensor_reduce(rowsum[:], e8t_v,
                                    axis=mybir.AxisListType.X, op=OP.add)

            # ---- big matvec: p_k = e @ vals[:, cols_k] (fp8 DR stream) --
            # sync HWDGE queue carries ONLY these blocks -> earliest start.
            p0 = pp.tile([1, 128], F32)
            v0 = blk0_sb[:, E8OFF:].rearrange("p (i j c) -> p i j c",
                                              j=2, c=128)
            for g in range(NB0):
                e2 = e8[:, :, 16 * g:16 * g + 1]
                nc.tensor.matmul(p0[:], e2, v0[:, g, :, :],
                                 start=(g == 0), stop=False,
                                 perf_mode=PM.DoubleRow)
            g = NB0
            for nb in BLOCKS:
                v = vp.tile([128, nb, 2, 128], F8, tag="v")
                src = d["vals_s"][:, (g - NB0) * 256:(g - NB0 + nb) * 256]
                nc.sync.dma_start(
                    v[:], src.rearrange("p (i j c) -> p i j c", j=2, c=128))
                for i in range(nb):
                    e2 = e8[:, :, 16 * g:16 * g + 1]
                    nc.tensor.matmul(p0[:], e2, v[:, i, :, :],
                                     start=False, stop=(g == NG - 1),
                                     perf_mode=PM.DoubleRow)
                    g += 1

            # ---- small chains, scheduled late so they gap-fill the PE ---
            with tc.tile_wait_until(0.007):
                # S = full softmax denominator via two tiny matmuls
                s1p = pp.tile([1, 1], F32, tag="s1")
                nc.tensor.matmul(s1p[:], rowsum[:], ones_col[:],
                                 start=True, stop=True)
                s1s = cp.tile([1, 1], F32)
                nc.vector.tensor_copy(s1s[:], s1p[:])
                sbp = pp.tile([128, 1], F32, tag="sb")
                nc.tensor.matmul(sbp[:], ones_row[:], s1s[:],
                                 start=True, stop=True)
                invS = cp.tile([128, 1], F32)
                nc.vector.reciprocal(invS[:], sbp[:])
            with tc.tile_wait_until(0.006):
                # preact slice: accumulate W_h2h[rows_k] @ h over 8 chunks
                # (scheduled into the PE's early DMA-ramp bubble; NOT late,
                # where its ~5us of fp16 matmuls would push out the DR tail)
                pre_a = pp.tile([1, 512], F32, tag="pre_a")
                pre_b = pp.tile([1, 128], F32, tag="pre_b")
                for c in range(8):
                    nc.tensor.matmul(pre_a[:], h8_sb[:, c:c + 1],
                                     wht_sb[:, c, 0:512],
                                     start=(c == 0), stop=(c == 7))
                    nc.tensor.matmul(pre_b[:], h8_sb[:, c:c + 1],
                                     wht_sb[:, c, 512:640],
                                     start=(c == 0), stop=(c == 7))
                row640 = cp.tile([1, 640], F16)
                nc.vector.tensor_copy(row640[0:1, 0:512], pre_a[:])
                nc.vector.tensor_copy(row640[0:1, 512:640], pre_b[:])
                psum_pre = pp.tile([128, 10], F16, tag="pre_t")
                for t in range(5):
                    # even f16 col = 4-byte-aligned PSUM write
                    nc.tensor.transpose(psum_pre[:, 2 * t:2 * t + 1],
                                        row640[0:1, t * 128:(t + 1) * 128],
                                        one16[:])
                prefull = cp.tile([128, 5], F32)
                nc.vector.tensor_add(
                    prefull[:].rearrange("p (c one) -> p c one", one=1),
                    psum_pre[:].rearrange("p (c two) -> p c two",
                                          two=2)[:, :, 0:1],
                    b5t_sb[:].rearrange("p (c one) -> p c one", one=1))
                th = cp.tile([128, 4], F32)
                nc.scalar.activation(th[:], prefull[:, 0:4], AF.Tanh,
                                     scale=0.5)
                gates = cp.tile([128, 4], F32)
                nc.vector.tensor_scalar(gates[:], th[:], 0.5, 0.5,
                                        OP.mult, OP.add)
                cnew = cp.tile([128, 1], F32)
                nc.scalar.activation(cnew[:], prefull[:, 4:5], AF.Tanh)
                t1 = cp.tile([128, 1], F32)
                nc.vector.tensor_mul(t1[:], gates[:, 0:1], c2t_sb[:])
                t2 = cp.tile([128, 1], F32)
                nc.vector.tensor_mul(t2[:], gates[:, 1:2], cnew[:])
                ct0 = cp.tile([128, 1], F32)
                nc.vector.tensor_add(ct0[:], t1[:], t2[:])

            # ---- LSTM tail: only r_t*m_t, c_t, h_t trail the stream -----
            p_row = cp.tile([1, 128], F32)
            nc.vector.tensor_copy(p_row[:], p0[:])
            pcol = pp.tile([128, 1], F32, tag="pcol")
            nc.tensor.transpose(pcol[:], p_row[:], ones1[:])
            mt = cp.tile([128, 1], F32)
            nc.scalar.activation(mt[:], pcol[:], AF.Tanh, scale=invS[:, 0:1])
            t3 = cp.tile([128, 1], F32)
            nc.vector.tensor_mul(t3[:], gates[:, 3:4], mt[:])
            out_sb = cp.tile([128, 10], F32)
            # tct = tanh(t3 + ct0) via the activation bias port; the c_t
            # output add runs in parallel on the vector engine
            tct = cp.tile([128, 1], F32)
            nc.scalar.activation(tct[:], t3[:], AF.Tanh, bias=ct0[:, 0:1])
            nc.vector.tensor_add(out_sb[:, 1:2], ct0[:], t3[:])
            # h_t in bf16 straight off the multiply (A2C needs bf16 anyway);
            # the f32 output copy overlaps the A2C matmuls
            ht_b = cp.tile([128, 1], BF16)
            nc.vector.tensor_mul(ht_b[:], gates[:, 2:3], tct[:])
            nc.vector.tensor_copy(out_sb[:, 0:1], ht_b[:])

            # ---- A2C partial, partition-parallel: q8[p, c] = -------------
            #      sum_j W_ih[c*128+p, c0+j] * h_t[c0+j]
            wihs_v = wihs_sb[:].rearrange("p (c j) -> p c j", j=128)
            q8 = pp.tile([128, 8], F32, tag="pre_a")
            for cc in range(8):
                nc.tensor.matmul(q8[:, cc:cc + 1], wihs_v[:, cc, :], ht_b[:],
                                 start=True, stop=True)
            nc.vector.tensor_copy(out_sb[:, 2:10], q8[:])
            nc.scalar.dma_start(out_hc[:], out_sb[:])

    nc.compile()
    return nc


def _get_nc():
    if "nc" not in _CACHE:
        _CACHE["nc"] = _build()
    return _CACHE["nc"]


def _prep_in_maps(x_t, h, c, keys, vals, W_i2h, b_i2h, W_h2h, b_h2h,
                  W_ih, b_ih, W_actor, b_actor, W_critic, b_critic, pick_arm):
    f = np.float32
    BF = ml_dtypes.bfloat16
    F8N = ml_dtypes.float8_e4m3
    x_t = np.asarray(x_t, f)
    h = np.asarray(h, f).reshape(-1)          # [H]
    c = np.asarray(c, f).reshape(-1)          # [H]
    keys = np.asarray(keys, f)
    vals = np.asarray(vals, f)
    W_h2h = np.asarray(W_h2h, f)
    W_ih = np.asarray(W_ih, f)

    pa = int(np.asarray(pick_arm))
    start = min(max(pa * RD, 0), IN_DIM - RD)  # jax dynamic_slice clamping
    q = x_t[0, start:start + RD]

    # host fold: cosine sims (like the baseline's key-norm fold), max-sub
    qn = np.linalg.norm(q)
    kn = np.linalg.norm(keys, axis=1)
    dots = (keys @ q) / np.maximum(kn * qn, 1e-8)
    dots = dots - dots.max()
    e_pad = np.zeros(PAD_D, f)
    e_pad[:D] = np.exp(dots)
    e8t = np.zeros((128, E8OFF), F8N)
    e8t[:, 0:CH] = e_pad.reshape(CH, 128).T.astype(F8N)

    b5 = (np.asarray(b_i2h, f) + np.asarray(b_h2h, f) + W_i2h @ x_t[0])
    b5m = b5.reshape(5, H)                    # [gate, h-dim]
    h8 = np.ascontiguousarray(h.reshape(8, 128).T).astype(np.float16)

    vals_pad = np.zeros((PAD_D, H), f)
    vals_pad[:D] = vals

    in_maps = []
    for k in range(N_CORES):
        c0 = k * 128
        vals_all = np.ascontiguousarray(
            vals_pad[:, c0:c0 + 128].reshape(NG, 2, 128, 128)
            .transpose(2, 0, 1, 3).reshape(128, NG * 2 * 128)).astype(F8N)
        blk0 = np.concatenate([e8t, vals_all[:, 0:NB0 * 256]], axis=1)
        vals_s = np.ascontiguousarray(vals_all[:, NB0 * 256:])
        # wht[p, cchunk, m=(g,j)] = W_h2h[g*H + c0 + j, cchunk*128 + p]
        rows = (np.arange(5)[:, None] * H + c0 + np.arange(128)[None, :]
                ).reshape(-1)                 # [640]
        wslice = W_h2h[rows]                  # [640, 1024]
        wht = np.ascontiguousarray(
            wslice.T.reshape(8, 128, 640).transpose(1, 0, 2)
            .reshape(128, 8 * 640)).astype(np.float16)
        b5t = np.ascontiguousarray(b5m[:, c0:c0 + 128].T)
        c2t = np.ascontiguousarray(c[c0:c0 + 128].reshape(128, 1))
        wihs = np.ascontiguousarray(W_ih[:, c0:c0 + 128].T).astype(BF)

        in_maps.append({
            "blk0": blk0,
            "vals_s": vals_s,
            "wht": wht,
            "h8": h8,
            "b5t": b5t,
            "c2t": c2t,
            "wihs": wihs,
        })
    return in_maps


def _postprocess(results, b_ih, b_actor, b_critic, W_actor, W_critic):
    f = np.float32
    h_t = np.empty(H, f)
    c_t = np.empty(H, f)
    for k, r in enumerate(results):
        h_t[k * 128:(k + 1) * 128] = r["out_hc"][:, 0]
        c_t[k * 128:(k + 1) * 128] = r["out_hc"][:, 1]
    # q8[p, c] = q_k[c*128 + p] -> flatten per core, sum across cores
    qsum = np.sum([np.asarray(r["out_hc"][:, 2:10], np.float64).T.reshape(-1)
                   for r in results], axis=0)
    hh = np.maximum(qsum + np.asarray(b_ih, np.float64), 0.0)
    logits = (np.asarray(W_actor, np.float64) @ hh
              + np.asarray(b_actor, np.float64))
    v = np.float32((np.asarray(W_critic, np.float64) @ hh
                    + np.asarray(b_critic, np.float64))[0])
    m = logits.max()
    ex = np.exp(logits - m)
    pi = (ex / ex.sum()).astype(f)
    a = int(np.argmax(np.log(pi) + GUMBEL))
    logp = np.float32(np.log(pi[a]))
    return np.concatenate([pi, [v], [logp], h_t, c_t]).astype(f)


def kernel(**inputs) -> np.ndarray:
    nc = _get_nc()
    in_maps = _prep_in_maps(**inputs)
    res = run_bass_kernel_spmd(
        nc, in_maps, core_ids=list(range(N_CORES)),
        **_CACHE.get("run_kwargs", {}))
    _CACHE["last_results"] = res
    return _postprocess(res.results, inputs["b_ih"], inputs["b_actor"],
                        inputs["b_critic"], inputs["W_actor"],
                        inputs["W_critic"])


# revision 45
# speedup vs baseline: 1.0790x; 1.0790x over previous
"""Trainium2 Bass kernel for CompositionalTwoArmedAgent (DND-LSTM A2C step).

Strategy (8 NeuronCores, SPMD, ZERO collectives):
  - DND vals table sharded COLUMN-wise: core k owns h-dims [128k, 128k+128)
    end-to-end -- its m_t slice, its 640 W_h2h rows, its gates, and its
    h_t / c_t slices.  No cross-core dependency, so no AllReduce: on this
    tunneled runtime the collective stack costs ~100us (entry barrier +
    33us mesh AR for 25KB), dwarfing the ~40us of real work.
  - Cosine sims are host-folded (like the baseline's key-norm fold): the
    device gets max-subtracted dots, does exp -> rowsum -> full-sum (two
    tiny PE matmuls; no gpsimd) -> fp8 DoubleRow weights.  dots arrive in
    two halves so exp/cast pipeline with the DMA.
  - vals stream in fp8e4 DoubleRow pairs (2 row-chunks of 128/partition):
    391 matmuls of FD=128, DMA-bound at ~350GB/s for 12.8MB/core.  The
    sync HWDGE queue carries ONLY the vals blocks so the stream starts at
    the queue-open tick; blocks taper at the end to cut the PE tail-lag.
  - LSTM preact slice = W_h2h[rows] @ h accumulated over 8 h-chunks; the
    x_t @ W_i2h part folds into the bias on the host.  The whole preact/
    gate chain is pushed late in the Tile schedule (tile_wait_until) so it
    gap-fills the PE instead of blocking the DR stream behind the slower
    wht DMA.
  - A2C: each core emits q_k = W_ih[:, cols_k] @ h_t[cols_k] (all GEMM
    flops stay on device); the host sums the 8 partials, applies
    relu/actor/critic/softmax/sampling -- same kind of post-processing the
    row-sharded baseline already did.
"""

import ml_dtypes
import numpy as np

import concourse.bacc as bacc
import concourse.bass as bass
import concourse.mybir as mybir
import concourse.tile as tile
from concourse.bass_utils import run_bass_kernel_spmd

N_CORES = 8
D, RD, H, IN_DIM = 100000, 10, 1024, 14
CH = 782               # 128-row chunks over D (padded to 100096)
NG = 391               # DoubleRow chunk pairs
NGA = 196              # pairs covered by the first dots half
PAD_D = CH * 128
# vals DMA blocks all ride the sync HWDGE queue (the two HWDGE queues share
# one SDMA engine pool, so splitting them halves per-queue rate).  DMA rate
# is per-partition-line-size bound: 64-pair blocks = 16KB lines sustain
# ~430GB/s while small blocks collapse to 80-250GB/s, so blocks are uniform
# 64 with one small remainder at the end (short wire + short PE lag).
NB0 = 32               # pairs fused into the blk0 transfer with the weights
E8OFF = 784            # e8t bytes (782) padded to 16B alignment in blk0
BLOCKS = [32, 32, 64, 64, 64, 32, 32, 32, 7]   # remaining pair blocks
assert NB0 + sum(BLOCKS) == NG
F32 = mybir.dt.float32
F8 = mybir.dt.float8e4
BF16 = mybir.dt.bfloat16
F16 = mybir.dt.float16

# jax.random.gumbel(jax.random.key(1), (2,), float32) — fixed constants of the
# reference's categorical sample (verified against jax.random.categorical).
GUMBEL = np.array([0.5325072, -0.01641824], np.float32)

_CACHE = {}


def _input_specs():
    return [
        # [exp(dots-max) fp8 tight (782B, padded to 784) | first NB0 pairs of
        # vals] fused into one big-line transfer so the weights ride at full
        # DMA rate instead of 25GB/s small-line rate.
        ("blk0", [128, E8OFF + NB0 * 256], F8),
        ("vals_s", [128, sum(BLOCKS) * 256], F8),  # remaining vals pairs
        ("wht", [128, 8 * 640], F16),         # W_h2h rows for this col slice
        ("h8", [128, 8], F16),                # full h, chunked
        ("b5t", [128, 5], F32),               # b_i2h + b_h2h + W_i2h @ x slice
        ("c2t", [128, 1], F32),               # c slice
        ("wihs", [128, 1024], BF16),          # W_ih[:, cols_k].T
    ]


def _build():
    nc = bacc.Bacc("TRN2", target_bir_lowering=False, debug=False,
                   num_devices=1)
    d = {name: nc.dram_tensor(name, shp, dt, kind="ExternalInput")
         for name, shp, dt in _input_specs()}
    # [h_t | c_t | q partial as [128, 8]] -- one DMA, 40B lines
    out_hc = nc.dram_tensor("out_hc", [128, 10], F32, kind="ExternalOutput")

    AF = mybir.ActivationFunctionType
    OP = mybir.AluOpType
    PM = mybir.MatmulPerfMode

    with tile.TileContext(nc) as tc:
        with (
            tc.tile_pool(name="const", bufs=1) as cp,
            tc.tile_pool(name="vals", bufs=5) as vp,
            tc.tile_pool(name="ps", bufs=1, space="PSUM") as pp,
        ):
            # ---- persistent loads, split by queue for priority ----------
            blk0_sb = cp.tile([128, E8OFF + NB0 * 256], F8)
            wht_sb = cp.tile([128, 8, 640], F16)
            h8_sb = cp.tile([128, 8], F16)
            b5t_sb = cp.tile([128, 5], F32)
            c2t_sb = cp.tile([128, 1], F32)
            wihs_sb = cp.tile([128, 1024], BF16)
            # fused weights+first-vals transfer heads the sync HWDGE queue
            nc.sync.dma_start(blk0_sb[:], d["blk0"][:])
            # preact weights on the otherwise-empty scalar HWDGE queue so
            # they land ~12us and the early preact slot can never stall
            nc.scalar.dma_start(
                wht_sb[:], d["wht"][:].rearrange("p (c m) -> p c m", m=640))
            nc.scalar.dma_start(h8_sb[:], d["h8"][:])
            # tail-only tensors on the gpsimd SWDGE queue (idle engine)
            nc.gpsimd.dma_start(b5t_sb[:], d["b5t"][:])
            nc.gpsimd.dma_start(c2t_sb[:], d["c2t"][:])
            nc.gpsimd.dma_start(wihs_sb[:], d["wihs"][:])

            one16 = cp.tile([1, 1], F16)
            nc.vector.memset(one16[:], 1.0)
            ones1 = cp.tile([1, 1], F32)
            nc.vector.memset(ones1[:], 1.0)
            ones_col = cp.tile([128, 1], F32)
            nc.vector.memset(ones_col[:], 1.0)
            ones_row = cp.tile([1, 128], F32)
            nc.vector.memset(ones_row[:], 1.0)

            # ---- softmax numerator weights: scatter into the DoubleRow
            # 16B-strided stationary layout (halves, so DR g=0 starts early)
            e8t_v = blk0_sb[:, 0:CH]
            e8 = cp.tile([128, 2, 16 * NG], F8)
            e8v = e8[:].rearrange("p j (g s) -> p j g s", s=16)
            erv = e8t_v.rearrange("p (g j one) -> p j g one", j=2, one=1)
            nc.vector.tensor_copy(e8v[:, :, 0:NGA, 0:1], erv[:, :, 0:NGA, :])
            nc.vector.tensor_copy(e8v[:, :, NGA:NG, 0:1], erv[:, :, NGA:NG, :])
            rowsum = cp.tile([128, 1], F32)
            nc.vector.tensor_reduce(rowsum[:], e8t_v,
                                    axis=mybir.AxisListType.X, op=OP.add)

            # ---- big matvec: p_k = e @ vals[:, cols_k] (fp8 DR stream) --
            # sync HWDGE queue carries ONLY these blocks -> earliest start.
            p0 = pp.tile([1, 128], F32)
            v0 = blk0_sb[:, E8OFF:].rearrange("p (i j c) -> p i j c",
                                              j=2, c=128)
            for g in range(NB0):
                e2 = e8[:, :, 16 * g:16 * g + 1]
                nc.tensor.matmul(p0[:], e2, v0[:, g, :, :],
                                 start=(g == 0), stop=False,
                                 perf_mode=PM.DoubleRow)
            g = NB0
            for nb in BLOCKS:
                v = vp.tile([128, nb, 2, 128], F8, tag="v")
                src = d["vals_s"][:, (g - NB0) * 256:(g - NB0 + nb) * 256]
                nc.sync.dma_start(
                    v[:], src.rearrange("p (i j c) -> p i j c", j=2, c=128))
                for i in range(nb):
                    e2 = e8[:, :, 16 * g:16 * g + 1]
                    nc.tensor.matmul(p0[:], e2, v[:, i, :, :],
                                     start=False, stop=(g == NG - 1),
                                     perf_mode=PM.DoubleRow)
                    g += 1

            # ---- small chains, scheduled late so they gap-fill the PE ---
            with tc.tile_wait_until(0.007):
                # S = full softmax denominator via two tiny matmuls
                s1p = pp.tile([1, 1], F32, tag="s1")
                nc.tensor.matmul(s1p[:], rowsum[:], ones_col[:],
                                 start=True, stop=True)
                s1s = cp.tile([1, 1], F32)
                nc.vector.tensor_copy(s1s[:], s1p[:])
                sbp = pp.tile([128, 1], F32, tag="sb")
                nc.tensor.matmul(sbp[:], ones_row[:], s1s[:],
                                 start=True, stop=True)
                invS = cp.tile([128, 1], F32)
                nc.vector.reciprocal(invS[:], sbp[:])
            # preact slice: accumulate W_h2h[rows_k] @ h over 8 chunks.
            # Sprinkled as 8 small pieces across early schedule slots so the
            # PE gap-fills DMA waits without any one slab stalling the
            # stream (a single 5-9us preact slab blocks tile releases and
            # starves the DMA).
            pre_a = pp.tile([1, 512], F32, tag="pre_a")
            pre_b = pp.tile([1, 128], F32, tag="pre_b")
            for c in range(8):
                with tc.tile_wait_until(0.003 + 0.001 * c):
                    nc.tensor.matmul(pre_a[:], h8_sb[:, c:c + 1],
                                     wht_sb[:, c, 0:512],
                                     start=(c == 0), stop=(c == 7))
                    nc.tensor.matmul(pre_b[:], h8_sb[:, c:c + 1],
                                     wht_sb[:, c, 512:640],
                                     start=(c == 0), stop=(c == 7))
            with tc.tile_wait_until(0.012):
                row640 = cp.tile([1, 640], F16)
                nc.vector.tensor_copy(row640[0:1, 0:512], pre_a[:])
                nc.vector.tensor_copy(row640[0:1, 512:640], pre_b[:])
                psum_pre = pp.tile([128, 10], F16, tag="pre_t")
                for t in range(5):
                    # even f16 col = 4-byte-aligned PSUM write
                    nc.tensor.transpose(psum_pre[:, 2 * t:2 * t + 1],
                                        row640[0:1, t * 128:(t + 1) * 128],
                                        one16[:])
                prefull = cp.tile([128, 5], F32)
                nc.vector.tensor_add(
                    prefull[:].rearrange("p (c one) -> p c one", one=1),
                    psum_pre[:].rearrange("p (c two) -> p c two",
                                          two=2)[:, :, 0:1],
                    b5t_sb[:].rearrange("p (c one) -> p c one", one=1))
                th = cp.tile([128, 4], F32)
                nc.scalar.activation(th[:], prefull[:, 0:4], AF.Tanh,
                                     scale=0.5)
                gates = cp.tile([128, 4], F32)
                nc.vector.tensor_scalar(gates[:], th[:], 0.5, 0.5,
                                        OP.mult, OP.add)
                cnew = cp.tile([128, 1], F32)
                nc.scalar.activation(cnew[:], prefull[:, 4:5], AF.Tanh)
                t1 = cp.tile([128, 1], F32)
                nc.vector.tensor_mul(t1[:], gates[:, 0:1], c2t_sb[:])
                t2 = cp.tile([128, 1], F32)
                nc.vector.tensor_mul(t2[:], gates[:, 1:2], cnew[:])
                ct0 = cp.tile([128, 1], F32)
                nc.vector.tensor_add(ct0[:], t1[:], t2[:])

            # ---- LSTM tail: only r_t*m_t, c_t, h_t trail the stream -----
            p_row = cp.tile([1, 128], F32)
            nc.vector.tensor_copy(p_row[:], p0[:])
            pcol = pp.tile([128, 1], F32, tag="pcol")
            nc.tensor.transpose(pcol[:], p_row[:], ones1[:])
            mt = cp.tile([128, 1], F32)
            nc.scalar.activation(mt[:], pcol[:], AF.Tanh, scale=invS[:, 0:1])
            t3 = cp.tile([128, 1], F32)
            nc.vector.tensor_mul(t3[:], gates[:, 3:4], mt[:])
            out_sb = cp.tile([128, 10], F32)
            # tct = tanh(t3 + ct0) via the activation bias port; the c_t
            # output add runs in parallel on the vector engine
            tct = cp.tile([128, 1], F32)
            nc.scalar.activation(tct[:], t3[:], AF.Tanh, bias=ct0[:, 0:1])
            nc.vector.tensor_add(out_sb[:, 1:2], ct0[:], t3[:])
            # h_t in bf16 straight off the multiply (A2C needs bf16 anyway);
            # the f32 output copy overlaps the A2C matmuls
            ht_b = cp.tile([128, 1], BF16)
            nc.vector.tensor_mul(ht_b[:], gates[:, 2:3], tct[:])
            nc.vector.tensor_copy(out_sb[:, 0:1], ht_b[:])

            # ---- A2C partial, partition-parallel: q8[p, c] = -------------
            #      sum_j W_ih[c*128+p, c0+j] * h_t[c0+j]
            wihs_v = wihs_sb[:].rearrange("p (c j) -> p c j", j=128)
            q8 = pp.tile([128, 8], F32, tag="pre_a")
            for cc in range(8):
                nc.tensor.matmul(q8[:, cc:cc + 1], wihs_v[:, cc, :], ht_b[:],
                                 start=True, stop=True)
            nc.vector.tensor_copy(out_sb[:, 2:10], q8[:])
            nc.scalar.dma_start(out_hc[:], out_sb[:])

    nc.compile()
    return nc


def _get_nc():
    if "nc" not in _CACHE:
        _CACHE["nc"] = _build()
    return _CACHE["nc"]


def _prep_in_maps(x_t, h, c, keys, vals, W_i2h, b_i2h, W_h2h, b_h2h,
                  W_ih, b_ih, W_actor, b_actor, W_critic, b_critic, pick_arm):
    f = np.float32
    BF = ml_dtypes.bfloat16
    F8N = ml_dtypes.float8_e4m3
    x_t = np.asarray(x_t, f)
    h = np.asarray(h, f).reshape(-1)          # [H]
    c = np.asarray(c, f).reshape(-1)          # [H]
    keys = np.asarray(keys, f)
    vals = np.asarray(vals, f)
    W_h2h = np.asarray(W_h2h, f)
    W_ih = np.asarray(W_ih, f)

    pa = int(np.asarray(pick_arm))
    start = min(max(pa * RD, 0), IN_DIM - RD)  # jax dynamic_slice clamping
    q = x_t[0, start:start + RD]

    # host fold: cosine sims (like the baseline's key-norm fold), max-sub
    qn = np.linalg.norm(q)
    kn = np.linalg.norm(keys, axis=1)
    dots = (keys @ q) / np.maximum(kn * qn, 1e-8)
    dots = dots - dots.max()
    e_pad = np.zeros(PAD_D, f)
    e_pad[:D] = np.exp(dots)
    e8t = np.zeros((128, E8OFF), F8N)
    e8t[:, 0:CH] = e_pad.reshape(CH, 128).T.astype(F8N)

    b5 = (np.asarray(b_i2h, f) + np.asarray(b_h2h, f) + W_i2h @ x_t[0])
    b5m = b5.reshape(5, H)                    # [gate, h-dim]
    h8 = np.ascontiguousarray(h.reshape(8, 128).T).astype(np.float16)

    vals_pad = np.zeros((PAD_D, H), f)
    vals_pad[:D] = vals

    in_maps = []
    for k in range(N_CORES):
        c0 = k * 128
        vals_all = np.ascontiguousarray(
            vals_pad[:, c0:c0 + 128].reshape(NG, 2, 128, 128)
            .transpose(2, 0, 1, 3).reshape(128, NG * 2 * 128)).astype(F8N)
        blk0 = np.concatenate([e8t, vals_all[:, 0:NB0 * 256]], axis=1)
        vals_s = np.ascontiguousarray(vals_all[:, NB0 * 256:])
        # wht[p, cchunk, m=(g,j)] = W_h2h[g*H + c0 + j, cchunk*128 + p]
        rows = (np.arange(5)[:, None] * H + c0 + np.arange(128)[None, :]
                ).reshape(-1)                 # [640]
        wslice = W_h2h[rows]                  # [640, 1024]
        wht = np.ascontiguousarray(
            wslice.T.reshape(8, 128, 640).transpose(1, 0, 2)
            .reshape(128, 8 * 640)).astype(np.float16)
        b5t = np.ascontiguousarray(b5m[:, c0:c0 + 128].T)
        c2t = np.ascontiguousarray(c[c0:c0 + 128].reshape(128, 1))
        wihs = np.ascontiguousarray(W_ih[:, c0:c0 + 128].T).astype(BF)

        in_maps.append({
            "blk0": blk0,
            "vals_s": vals_s,
            "wht": wht,
            "h8": h8,
            "b5t": b5t,
            "c2t": c2t,
            "wihs": wihs,
        })
    return in_maps


def _postprocess(results, b_ih, b_actor, b_critic, W_actor, W_critic):
    f = np.float32
    h_t = np.empty(H, f)
    c_t = np.empty(H, f)
    for k, r in enumerate(results):
        h_t[k * 128:(k + 1) * 128] = r["out_hc"][:, 0]
        c_t[k * 128:(k + 1) * 128] = r["out_hc"][:, 1]
    # q8[p, c] = q_k[c*128 + p] -> flatten per core, sum across cores
    qsum = np.sum([np.asarray(r["out_hc"][:, 2:10], np.float64).T.reshape(-1)
                   for r in results], axis=0)
    hh = np.maximum(qsum + np.asarray(b_ih, np.float64), 0.0)
    logits = (np.asarray(W_actor, np.float64) @ hh
              + np.asarray(b_actor, np.float64))
    v = np.float32((np.asarray(W_critic, np.float64) @ hh
                    + np.asarray(b_critic, np.float64))[0])
    m = logits.max()
    ex = np.exp(logits - m)
    pi = (ex / ex.sum()).astype(f)
    a = int(np.argmax(np.log(pi) + GUMBEL))
    logp = np.float32(np.log(pi[a]))
    return np.concatenate([pi, [v], [logp], h_t, c_t]).astype(f)


def kernel(**inputs) -> np.ndarray:
    nc = _get_nc()
    in_maps = _prep_in_maps(**inputs)
    res = run_bass_kernel_spmd(
        nc, in_maps, core_ids=list(range(N_CORES)),
        **_CACHE.get("run_kwargs", {}))
    _CACHE["last_results"] = res
    return _postprocess(res.results, inputs["b_ih"], inputs["b_actor"],
                        inputs["b_critic"], inputs["W_actor"],
                        inputs["W_critic"])


# revision 46
# speedup vs baseline: 1.2077x; 1.1193x over previous
"""Trainium2 Bass kernel for CompositionalTwoArmedAgent (DND-LSTM A2C step).

Strategy (8 NeuronCores, SPMD, ZERO collectives):
  - DND vals table sharded COLUMN-wise: core k owns h-dims [128k, 128k+128)
    end-to-end -- its m_t slice, its 640 W_h2h rows, its gates, and its
    h_t / c_t slices.  No cross-core dependency, so no AllReduce: on this
    tunneled runtime the collective stack costs ~100us (entry barrier +
    33us mesh AR for 25KB), dwarfing the ~40us of real work.
  - Cosine sims are host-folded (like the baseline's key-norm fold): the
    device gets max-subtracted dots, does exp -> rowsum -> full-sum (two
    tiny PE matmuls; no gpsimd) -> fp8 DoubleRow weights.  dots arrive in
    two halves so exp/cast pipeline with the DMA.
  - vals stream in fp8e4 DoubleRow pairs (2 row-chunks of 128/partition):
    391 matmuls of FD=128, DMA-bound at ~350GB/s for 12.8MB/core.  The
    sync HWDGE queue carries ONLY the vals blocks so the stream starts at
    the queue-open tick; blocks taper at the end to cut the PE tail-lag.
  - LSTM preact slice = W_h2h[rows] @ h accumulated over 8 h-chunks; the
    x_t @ W_i2h part folds into the bias on the host.  The whole preact/
    gate chain is pushed late in the Tile schedule (tile_wait_until) so it
    gap-fills the PE instead of blocking the DR stream behind the slower
    wht DMA.
  - A2C: each core emits q_k = W_ih[:, cols_k] @ h_t[cols_k] (all GEMM
    flops stay on device); the host sums the 8 partials, applies
    relu/actor/critic/softmax/sampling -- same kind of post-processing the
    row-sharded baseline already did.
"""

import ml_dtypes
import numpy as np

import concourse.bacc as bacc
import concourse.bass as bass
import concourse.mybir as mybir
import concourse.tile as tile
from concourse.bass_utils import run_bass_kernel_spmd

N_CORES = 8
D, RD, H, IN_DIM = 100000, 10, 1024, 14
CH = 782               # 128-row chunks over D (padded to 100096)
NG = 391               # DoubleRow chunk pairs
NGA = 196              # pairs covered by the first dots half
PAD_D = CH * 128
# vals DMA blocks all ride the sync HWDGE queue (the two HWDGE queues share
# one SDMA engine pool, so splitting them halves per-queue rate).  DMA rate
# is per-partition-line-size bound: 64-pair blocks = 16KB lines sustain
# ~430GB/s while small blocks collapse to 80-250GB/s, so blocks are uniform
# 64 with one small remainder at the end (short wire + short PE lag).
NB0 = 32               # pairs fused into the blk0 transfer with the weights
E8OFF = 784            # e8t bytes (782) padded to 16B alignment in blk0
BLOCKS = [32, 32, 64, 64, 64, 32, 32, 32, 7]   # remaining pair blocks
assert NB0 + sum(BLOCKS) == NG
F32 = mybir.dt.float32
F8 = mybir.dt.float8e4
BF16 = mybir.dt.bfloat16
F16 = mybir.dt.float16

# jax.random.gumbel(jax.random.key(1), (2,), float32) — fixed constants of the
# reference's categorical sample (verified against jax.random.categorical).
GUMBEL = np.array([0.5325072, -0.01641824], np.float32)

_CACHE = {}


def _input_specs():
    return [
        # [exp(dots-max) fp8 tight (782B, padded to 784) | first NB0 pairs of
        # vals] fused into one big-line transfer so the weights ride at full
        # DMA rate instead of 25GB/s small-line rate.
        ("blk0", [128, E8OFF + NB0 * 256], F8),
        ("vals_s", [128, sum(BLOCKS) * 256], F8),  # remaining vals pairs
        ("wht", [128, 8 * 640], F16),         # W_h2h rows for this col slice
        ("h8", [128, 8], F16),                # full h, chunked
        ("b5t", [128, 5], F32),               # b_i2h + b_h2h + W_i2h @ x slice
        ("c2t", [128, 1], F32),               # c slice
        ("wihs", [128, 1024], BF16),          # W_ih[:, cols_k].T
    ]


def _build():
    nc = bacc.Bacc("TRN2", target_bir_lowering=False, debug=False,
                   num_devices=1)
    d = {name: nc.dram_tensor(name, shp, dt, kind="ExternalInput")
         for name, shp, dt in _input_specs()}
    # [h_t | c_t | q partial as [128, 8]] -- one DMA, 40B lines
    out_hc = nc.dram_tensor("out_hc", [128, 10], F32, kind="ExternalOutput")

    AF = mybir.ActivationFunctionType
    OP = mybir.AluOpType
    PM = mybir.MatmulPerfMode

    with tile.TileContext(nc) as tc:
        with (
            tc.tile_pool(name="const", bufs=1) as cp,
            tc.tile_pool(name="vals", bufs=5) as vp,
            tc.tile_pool(name="ps", bufs=1, space="PSUM") as pp,
        ):
            # ---- persistent loads, split by queue for priority ----------
            blk0_sb = cp.tile([128, E8OFF + NB0 * 256], F8)
            wht_sb = cp.tile([128, 8, 640], F16)
            h8_sb = cp.tile([128, 8], F16)
            b5t_sb = cp.tile([128, 5], F32)
            c2t_sb = cp.tile([128, 1], F32)
            wihs_sb = cp.tile([128, 1024], BF16)
            # fused weights+first-vals transfer heads the sync HWDGE queue
            nc.sync.dma_start(blk0_sb[:], d["blk0"][:])
            # preact weights on the otherwise-empty scalar HWDGE queue so
            # they land ~12us and the early preact slots can never stall;
            # split per h-chunk so preact piece c only waits for chunk c
            # (the scheduler keys the static order off simulated arrival)
            nc.scalar.dma_start(h8_sb[:], d["h8"][:])
            whtd = d["wht"][:].rearrange("p (c m) -> p c m", m=640)
            for c in range(8):
                nc.scalar.dma_start(wht_sb[:, c, :], whtd[:, c, :])
            # tail-only tensors on the gpsimd SWDGE queue (idle engine)
            nc.gpsimd.dma_start(b5t_sb[:], d["b5t"][:])
            nc.gpsimd.dma_start(c2t_sb[:], d["c2t"][:])
            nc.gpsimd.dma_start(wihs_sb[:], d["wihs"][:])

            one16 = cp.tile([1, 1], F16)
            nc.vector.memset(one16[:], 1.0)
            ones1 = cp.tile([1, 1], F32)
            nc.vector.memset(ones1[:], 1.0)
            ones_col = cp.tile([128, 1], F32)
            nc.vector.memset(ones_col[:], 1.0)
            ones_row = cp.tile([1, 128], F32)
            nc.vector.memset(ones_row[:], 1.0)

            # ---- softmax numerator weights: scatter into the DoubleRow
            # 16B-strided stationary layout (halves, so DR g=0 starts early)
            e8t_v = blk0_sb[:, 0:CH]
            e8 = cp.tile([128, 2, 16 * NG], F8)
            e8v = e8[:].rearrange("p j (g s) -> p j g s", s=16)
            erv = e8t_v.rearrange("p (g j one) -> p j g one", j=2, one=1)
            nc.vector.tensor_copy(e8v[:, :, 0:NGA, 0:1], erv[:, :, 0:NGA, :])
            nc.vector.tensor_copy(e8v[:, :, NGA:NG, 0:1], erv[:, :, NGA:NG, :])
            rowsum = cp.tile([128, 1], F32)
            nc.vector.tensor_reduce(rowsum[:], e8t_v,
                                    axis=mybir.AxisListType.X, op=OP.add)

            # ---- big matvec: p_k = e @ vals[:, cols_k] (fp8 DR stream) --
            # sync HWDGE queue carries ONLY these blocks -> earliest start.
            p0 = pp.tile([1, 128], F32)
            v0 = blk0_sb[:, E8OFF:].rearrange("p (i j c) -> p i j c",
                                              j=2, c=128)
            for g in range(NB0):
                e2 = e8[:, :, 16 * g:16 * g + 1]
                nc.tensor.matmul(p0[:], e2, v0[:, g, :, :],
                                 start=(g == 0), stop=False,
                                 perf_mode=PM.DoubleRow)
            g = NB0
            for nb in BLOCKS:
                v = vp.tile([128, nb, 2, 128], F8, tag="v")
                src = d["vals_s"][:, (g - NB0) * 256:(g - NB0 + nb) * 256]
                nc.sync.dma_start(
                    v[:], src.rearrange("p (i j c) -> p i j c", j=2, c=128))
                for i in range(nb):
                    e2 = e8[:, :, 16 * g:16 * g + 1]
                    nc.tensor.matmul(p0[:], e2, v[:, i, :, :],
                                     start=False, stop=(g == NG - 1),
                                     perf_mode=PM.DoubleRow)
                    g += 1

            # ---- small chains, scheduled late so they gap-fill the PE ---
            with tc.tile_wait_until(0.007):
                # S = full softmax denominator via two tiny matmuls
                s1p = pp.tile([1, 1], F32, tag="s1")
                nc.tensor.matmul(s1p[:], rowsum[:], ones_col[:],
                                 start=True, stop=True)
                s1s = cp.tile([1, 1], F32)
                nc.vector.tensor_copy(s1s[:], s1p[:])
                sbp = pp.tile([128, 1], F32, tag="sb")
                nc.tensor.matmul(sbp[:], ones_row[:], s1s[:],
                                 start=True, stop=True)
                invS = cp.tile([128, 1], F32)
                nc.vector.reciprocal(invS[:], sbp[:])
            # preact slice: accumulate W_h2h[rows_k] @ h over 8 chunks.
            # Sprinkled as 8 small pieces across early schedule slots so the
            # PE gap-fills DMA waits without any one slab stalling the
            # stream (a single 5-9us preact slab blocks tile releases and
            # starves the DMA).
            pre_a = pp.tile([1, 512], F32, tag="pre_a")
            pre_b = pp.tile([1, 128], F32, tag="pre_b")
            for c in range(8):
                with tc.tile_wait_until(0.003 + 0.001 * c):
                    nc.tensor.matmul(pre_a[:], h8_sb[:, c:c + 1],
                                     wht_sb[:, c, 0:512],
                                     start=(c == 0), stop=(c == 7))
                    nc.tensor.matmul(pre_b[:], h8_sb[:, c:c + 1],
                                     wht_sb[:, c, 512:640],
                                     start=(c == 0), stop=(c == 7))
            with tc.tile_wait_until(0.012):
                row640 = cp.tile([1, 640], F16)
                nc.vector.tensor_copy(row640[0:1, 0:512], pre_a[:])
                nc.vector.tensor_copy(row640[0:1, 512:640], pre_b[:])
                psum_pre = pp.tile([128, 10], F16, tag="pre_t")
                for t in range(5):
                    # even f16 col = 4-byte-aligned PSUM write
                    nc.tensor.transpose(psum_pre[:, 2 * t:2 * t + 1],
                                        row640[0:1, t * 128:(t + 1) * 128],
                                        one16[:])
                prefull = cp.tile([128, 5], F32)
                nc.vector.tensor_add(
                    prefull[:].rearrange("p (c one) -> p c one", one=1),
                    psum_pre[:].rearrange("p (c two) -> p c two",
                                          two=2)[:, :, 0:1],
                    b5t_sb[:].rearrange("p (c one) -> p c one", one=1))
                th = cp.tile([128, 4], F32)
                nc.scalar.activation(th[:], prefull[:, 0:4], AF.Tanh,
                                     scale=0.5)
                gates = cp.tile([128, 4], F32)
                nc.vector.tensor_scalar(gates[:], th[:], 0.5, 0.5,
                                        OP.mult, OP.add)
                cnew = cp.tile([128, 1], F32)
                nc.scalar.activation(cnew[:], prefull[:, 4:5], AF.Tanh)
                t1 = cp.tile([128, 1], F32)
                nc.vector.tensor_mul(t1[:], gates[:, 0:1], c2t_sb[:])
                t2 = cp.tile([128, 1], F32)
                nc.vector.tensor_mul(t2[:], gates[:, 1:2], cnew[:])
                ct0 = cp.tile([128, 1], F32)
                nc.vector.tensor_add(ct0[:], t1[:], t2[:])

            # ---- LSTM tail: only r_t*m_t, c_t, h_t trail the stream -----
            p_row = cp.tile([1, 128], F32)
            nc.vector.tensor_copy(p_row[:], p0[:])
            pcol = pp.tile([128, 1], F32, tag="pcol")
            nc.tensor.transpose(pcol[:], p_row[:], ones1[:])
            mt = cp.tile([128, 1], F32)
            nc.scalar.activation(mt[:], pcol[:], AF.Tanh, scale=invS[:, 0:1])
            t3 = cp.tile([128, 1], F32)
            nc.vector.tensor_mul(t3[:], gates[:, 3:4], mt[:])
            out_sb = cp.tile([128, 10], F32)
            # tct = tanh(t3 + ct0) via the activation bias port; the c_t
            # output add runs in parallel on the vector engine
            tct = cp.tile([128, 1], F32)
            nc.scalar.activation(tct[:], t3[:], AF.Tanh, bias=ct0[:, 0:1])
            nc.vector.tensor_add(out_sb[:, 1:2], ct0[:], t3[:])
            # h_t in bf16 straight off the multiply (A2C needs bf16 anyway);
            # the f32 output copy overlaps the A2C matmuls
            ht_b = cp.tile([128, 1], BF16)
            nc.vector.tensor_mul(ht_b[:], gates[:, 2:3], tct[:])
            nc.vector.tensor_copy(out_sb[:, 0:1], ht_b[:])

            # ---- A2C partial, partition-parallel: q8[p, c] = -------------
            #      sum_j W_ih[c*128+p, c0+j] * h_t[c0+j]
            wihs_v = wihs_sb[:].rearrange("p (c j) -> p c j", j=128)
            q8 = pp.tile([128, 8], F32, tag="pre_a")
            for cc in range(8):
                nc.tensor.matmul(q8[:, cc:cc + 1], wihs_v[:, cc, :], ht_b[:],
                                 start=True, stop=True)
            nc.vector.tensor_copy(out_sb[:, 2:10], q8[:])
            nc.scalar.dma_start(out_hc[:], out_sb[:])

    nc.compile()
    return nc


def _get_nc():
    if "nc" not in _CACHE:
        _CACHE["nc"] = _build()
    return _CACHE["nc"]


def _prep_in_maps(x_t, h, c, keys, vals, W_i2h, b_i2h, W_h2h, b_h2h,
                  W_ih, b_ih, W_actor, b_actor, W_critic, b_critic, pick_arm):
    f = np.float32
    BF = ml_dtypes.bfloat16
    F8N = ml_dtypes.float8_e4m3
    x_t = np.asarray(x_t, f)
    h = np.asarray(h, f).reshape(-1)          # [H]
    c = np.asarray(c, f).reshape(-1)          # [H]
    keys = np.asarray(keys, f)
    vals = np.asarray(vals, f)
    W_h2h = np.asarray(W_h2h, f)
    W_ih = np.asarray(W_ih, f)

    pa = int(np.asarray(pick_arm))
    start = min(max(pa * RD, 0), IN_DIM - RD)  # jax dynamic_slice clamping
    q = x_t[0, start:start + RD]

    # host fold: cosine sims (like the baseline's key-norm fold), max-sub
    qn = np.linalg.norm(q)
    kn = np.linalg.norm(keys, axis=1)
    dots = (keys @ q) / np.maximum(kn * qn, 1e-8)
    dots = dots - dots.max()
    e_pad = np.zeros(PAD_D, f)
    e_pad[:D] = np.exp(dots)
    e8t = np.zeros((128, E8OFF), F8N)
    e8t[:, 0:CH] = e_pad.reshape(CH, 128).T.astype(F8N)

    b5 = (np.asarray(b_i2h, f) + np.asarray(b_h2h, f) + W_i2h @ x_t[0])
    b5m = b5.reshape(5, H)                    # [gate, h-dim]
    h8 = np.ascontiguousarray(h.reshape(8, 128).T).astype(np.float16)

    vals_pad = np.zeros((PAD_D, H), f)
    vals_pad[:D] = vals

    in_maps = []
    for k in range(N_CORES):
        c0 = k * 128
        vals_all = np.ascontiguousarray(
            vals_pad[:, c0:c0 + 128].reshape(NG, 2, 128, 128)
            .transpose(2, 0, 1, 3).reshape(128, NG * 2 * 128)).astype(F8N)
        blk0 = np.concatenate([e8t, vals_all[:, 0:NB0 * 256]], axis=1)
        vals_s = np.ascontiguousarray(vals_all[:, NB0 * 256:])
        # wht[p, cchunk, m=(g,j)] = W_h2h[g*H + c0 + j, cchunk*128 + p]
        rows = (np.arange(5)[:, None] * H + c0 + np.arange(128)[None, :]
                ).reshape(-1)                 # [640]
        wslice = W_h2h[rows]                  # [640, 1024]
        wht = np.ascontiguousarray(
            wslice.T.reshape(8, 128, 640).transpose(1, 0, 2)
            .reshape(128, 8 * 640)).astype(np.float16)
        b5t = np.ascontiguousarray(b5m[:, c0:c0 + 128].T)
        c2t = np.ascontiguousarray(c[c0:c0 + 128].reshape(128, 1))
        wihs = np.ascontiguousarray(W_ih[:, c0:c0 + 128].T).astype(BF)

        in_maps.append({
            "blk0": blk0,
            "vals_s": vals_s,
            "wht": wht,
            "h8": h8,
            "b5t": b5t,
            "c2t": c2t,
            "wihs": wihs,
        })
    return in_maps


def _postprocess(results, b_ih, b_actor, b_critic, W_actor, W_critic):
    f = np.float32
    h_t = np.empty(H, f)
    c_t = np.empty(H, f)
    for k, r in enumerate(results):
        h_t[k * 128:(k + 1) * 128] = r["out_hc"][:, 0]
        c_t[k * 128:(k + 1) * 128] = r["out_hc"][:, 1]
    # q8[p, c] = q_k[c*128 + p] -> flatten per core, sum across cores
    qsum = np.sum([np.asarray(r["out_hc"][:, 2:10], np.float64).T.reshape(-1)
                   for r in results], axis=0)
    hh = np.maximum(qsum + np.asarray(b_ih, np.float64), 0.0)
    logits = (np.asarray(W_actor, np.float64) @ hh
              + np.asarray(b_actor, np.float64))
    v = np.float32((np.asarray(W_critic, np.float64) @ hh
                    + np.asarray(b_critic, np.float64))[0])
    m = logits.max()
    ex = np.exp(logits - m)
    pi = (ex / ex.sum()).astype(f)
    a = int(np.argmax(np.log(pi) + GUMBEL))
    logp = np.float32(np.log(pi[a]))
    return np.concatenate([pi, [v], [logp], h_t, c_t]).astype(f)


def kernel(**inputs) -> np.ndarray:
    nc = _get_nc()
    in_maps = _prep_in_maps(**inputs)
    res = run_bass_kernel_spmd(
        nc, in_maps, core_ids=list(range(N_CORES)),
        **_CACHE.get("run_kwargs", {}))
    _CACHE["last_results"] = res
    return _postprocess(res.results, inputs["b_ih"], inputs["b_actor"],
                        inputs["b_critic"], inputs["W_actor"],
                        inputs["W_critic"])
